# revision 42
# baseline (speedup 1.0000x reference)
"""Dual (global + local-masked) BERT self-attention on 8 Trainium2 NeuronCores.

Problem: B=2, S=2048, H=1024, NH=16 heads of DH=64.
  q/k/v = hidden @ W{q,k,v}.T + b ; scores = q k^T / 8
  probs_g = softmax(scores + attention_mask)         (additive, zeros in spec)
  probs_l = softmax(scores + (-inf where local_mask==0))
  out     = gate * (probs_l @ v) + (1-gate) * (probs_g @ v)

Sharding: 32 (batch, head) pairs -> 4 heads per core (core c: batch c//4,
heads 4*(c%4)..+4). Each core computes its heads' projections + dual
attention independently; no collectives.

Per-core kernel (all bf16 matmuls, f32 PSUM):
  - DMA issue spread across SP/ACT HWDGE queues (weights before X, mask
    split SP/ACT) so the first projection matmul starts ~3us in; per-slice
    input tiles keep dependencies at single-DMA granularity.
  - Q^T,K^T [128 dims (head pair), S] and natural-layout V (+ ones column)
    on PE; Q/K bias-add epilogue on DVE, V PSUM->SBUF copy on DVE, keeping
    ACT free for exp.
  - delayed-ctx software pipeline over (head, 1024-query chunk) blocks:
    block i computes chunk i's scores^T [128 keys, 1024 q] on PE (K=64)
    and e = exp(scores) on ACT (bf16, shared by both branches) into a
    20-deep SBUF e-ring, interleaved PER KEY TILE with chunk i-1's
    el = e * mask (DVE) and 4 flat ctx matmuls [65, 512] (lhsT =
    [V | ones], 512-col streams; denominators ride PSUM row 64 free).
    The ctx matmuls read fully materialized e tiles, so the PE never
    waits on the exp->mask chain and chunks have no fill/drain. The
    e-ring's 40KB comes from scoping X^T tiles to the projection phase.
  - normalize/combine epilogue pipelined two blocks deep in two stages:
      head (block end): 2 ACT copies release ctx PSUM; tiny DMAs move
        denominator row 64 -> partition 0 and fetch gate rows (SP queue).
      stage A (next block, t==2): one DVE reciprocal + one DVE gate-mul ->
        [1, 2, qcw] coefficients; 2 Pool partition_broadcasts (1-input Q7
        ops run at line rate; 2-input Pool ops do NOT - measured on HW).
      stage B (next block, t==6): 2 DVE muls + 1 DVE add combine the two
        branches with broadcast coefficients; bf16 out DMA.
  - PSUM: scores 2x[128,1024] double-buffered (4 banks) + ctx_l/ctx_g
    (4 banks); one accumulation group per 2KB bank (hardware zero-region).
Output per core: [256 dims, S] bf16 (host casts to f32 and transposes).
No max-subtraction in softmax: scores are O(+-5), exp is safe and softmax
is shift-invariant; matches the reference to ~5e-3 (gate < 2e-2).
Measured (repeat-slope r=1 vs r=32, device-resident inputs, median over
25 interleaved rounds): 213-260us/core across runs (cleanest window:
med 213/p25 216us) vs ~374us for the staged baseline on the same
protocol; CoreSim cost model: 247us single-shot, 224us marginal rep,
PE busy 206us = 84%. PE 206us is the bf16 floor for this algorithm
(fp8 fails the 2e-2 accuracy gate on every operand, measured; the
scores contraction K=64 < 128 is inherent to DH=64).
"""

import sys

sys.path.insert(0, "/opt/trn_rl_repo")

import numpy as np
import ml_dtypes

B, S, H, NH, DH = 2, 2048, 1024, 16, 64
NCORES = 8
HPC = 4          # heads per core
MPC = HPC // 2   # head pairs per core
QC = 1024        # query chunk (free dim of scores psum)
NQC = S // QC
NQB = S // 128   # 128-query blocks
QBC = QC // 128  # query blocks per chunk
KT = S // 128    # key tiles
XT_T = H // 128  # X^T k-tiles for projections

_BUILT = {}


def _build(use_em: bool, repeat: int = 1, has_b: bool = False,
           qcw: int = QC, scbufs: int = 2, ebufs: int = 8):
    from contextlib import ExitStack

    import concourse.mybir as mybir
    from concourse import bacc, tile

    f32 = mybir.dt.float32
    bf16 = mybir.dt.bfloat16
    AF = mybir.ActivationFunctionType
    OP = mybir.AluOpType

    qbc = qcw // 128

    nc = bacc.Bacc("TRN2", target_bir_lowering=False, debug=False)

    xt_d = nc.dram_tensor("xt", [H, S], bf16, kind="ExternalInput").ap()
    wq_d = nc.dram_tensor("wq", [H, 256], bf16, kind="ExternalInput").ap()
    wk_d = nc.dram_tensor("wk", [H, 256], bf16, kind="ExternalInput").ap()
    wv_d = nc.dram_tensor("wv", [H, 256], bf16, kind="ExternalInput").ap()
    bqk_d = nc.dram_tensor("bqk", [2, 256], f32, kind="ExternalInput").ap()
    bv_d = nc.dram_tensor("bv", [1, 256], bf16, kind="ExternalInput").ap()
    msk_d = nc.dram_tensor("msk", [KT, 128, S], bf16, kind="ExternalInput").ap()
    # gt[h, r, q]: head h, r = (gate_h, 1-gate_h)
    gt_d = nc.dram_tensor("gt", [HPC, 2, S], bf16, kind="ExternalInput").ap()
    if use_em:
        em_d = nc.dram_tensor("em", [KT, 128], f32, kind="ExternalInput").ap()
    out_d = nc.dram_tensor("out", [HPC * DH, S], bf16, kind="ExternalOutput").ap()

    with tile.TileContext(nc) as tc, ExitStack() as ctx:
        big = ctx.enter_context(tc.tile_pool(name="big", bufs=1))

        # inputs the projections need come first; mask can arrive during proj
        w_sbs = {}
        for nm, d in (("wq", wq_d), ("wk", wk_d), ("wv", wv_d)):
            w_sb = big.tile([128, XT_T, 256], bf16, name=f"{nm}_sb")
            for t in range(XT_T):
                # ACT queue: issues in parallel with xt issues on SP
                nc.scalar.dma_start(w_sb[:, t, :], d[t * 128:(t + 1) * 128, :])
            w_sbs[nm] = w_sb
        bqk_sb = big.tile([128, 2, 2], f32, name="bqk_sb")
        nc.sync.dma_start(bqk_sb, bqk_d.rearrange("c (t p) -> p c t", p=128))
        bv_sb = big.tile([1, 256], bf16, name="bv_sb")
        nc.sync.dma_start(bv_sb, bv_d)
        msk_ts = []
        for t in range(KT):
            msk_t = big.tile([128, S], bf16, name=f"msk{t}_sb")
            # ACT HWDGE queue: keeps SP free for the per-rep xt loads
            nc.scalar.dma_start(msk_t, msk_d[t])
            msk_ts.append(msk_t)
        if use_em:
            em_sb = big.tile([128, KT], f32, name="em_sb")
            nc.sync.dma_start(em_sb, em_d.rearrange("t p -> p t"))

        ones_r = big.tile([1, 128], bf16, name="ones_r")
        nc.vector.memset(ones_r, 1.0)

        qt_sb = big.tile([128, MPC, S], bf16, name="qt_sb")
        kt_sb = big.tile([128, MPC, S], bf16, name="kt_sb")
        v_ts = []
        for t in range(KT):
            v_t = big.tile([128, HPC, 65], bf16, name=f"v{t}_sb")
            nc.vector.memset(v_t[:, :, 64:65], 1.0)
            v_ts.append(v_t)

        for _rep in range(repeat):
            # ---- projections: Q^T, K^T (transposed), V (natural) ----
            # xt lives only through the projections; its 32KB/partition is
            # reclaimed for the e-ring during attention
            with tc.tile_pool(name="px", bufs=1) as px, \
                 tc.tile_pool(name="pproj", bufs=2, space="PSUM") as pproj:
              for _one in range(1):
                xt_ts = []
                for t in range(XT_T):
                    xt_t = px.tile([128, S], bf16, name=f"xt{t}_sb")
                    # rep 0: all on SP (ACT is busy issuing w+mask); later
                    # reps: even tiles via ACT (its DMA path is empty at rep
                    # boundaries, SP still drains epilogue DMAs), odd via SP
                    eng = nc.sync if (_rep == 0 or t % 2 == 1) else nc.scalar
                    eng.dma_start(xt_t, xt_d[t * 128:(t + 1) * 128, :])
                    xt_ts.append(xt_t)
                for m in range(MPC):
                    for ci, (wn, dst) in enumerate((("wq", qt_sb), ("wk", kt_sb))):
                        w_sb = w_sbs[wn]
                        for nq in range(S // 1024):
                            ps = pproj.tile([128, 1024], f32, tag="pp")
                            for t in range(XT_T):
                                for hlf in range(2):
                                    nc.tensor.matmul(
                                        ps[:, hlf * 512:(hlf + 1) * 512],
                                        lhsT=w_sb[:, t, m * 128:(m + 1) * 128],
                                        rhs=xt_ts[t][:, nq * 1024 + hlf * 512:
                                                     nq * 1024 + (hlf + 1) * 512],
                                        start=(t == 0),
                                        stop=(t == XT_T - 1),
                                    )
                            # bias-add + f32->bf16 on DVE (keeps ACT free)
                            nc.vector.tensor_scalar_add(
                                dst[:, m, nq * 1024:(nq + 1) * 1024], ps,
                                bqk_sb[:, ci, m:m + 1],
                            )
                for st in range(KT):
                    ps = pproj.tile([128, 256], f32, tag="pv")
                    for t in range(XT_T):
                        nc.tensor.matmul(
                            ps,
                            lhsT=xt_ts[t][:, st * 128:(st + 1) * 128],
                            rhs=w_sbs["wv"][:, t, :],
                            start=(t == 0),
                            stop=(t == XT_T - 1 and not has_b),
                        )
                    if has_b:
                        nc.tensor.matmul(
                            ps, lhsT=ones_r, rhs=bv_sb, start=False, stop=True
                        )
                    # PSUM->SBUF copy on DVE (keeps ACT free for exp)
                    nc.vector.tensor_copy(
                        v_ts[st][:, :, 0:64],
                        ps.rearrange("p (h d) -> p h d", h=HPC),
                    )

            # ---- dual attention ----
            att_ctx = ExitStack()
            psc = att_ctx.enter_context(
                tc.tile_pool(name="psc", bufs=scbufs, space="PSUM"))
            pctx = att_ctx.enter_context(
                tc.tile_pool(name="pctx", bufs=1, space="PSUM"))
            pe = att_ctx.enter_context(tc.tile_pool(name="pe", bufs=KT + 4))
            pel = att_ctx.enter_context(tc.tile_pool(name="pel", bufs=4))
            pc = att_ctx.enter_context(tc.tile_pool(name="pc", bufs=1))
            pt = att_ctx.enter_context(tc.tile_pool(name="pt", bufs=1))
            po = att_ctx.enter_context(tc.tile_pool(name="po", bufs=2))

            pending = [None, None]

            def _chain_a(cts2, s2_, gtt):
                def run():
                    r2 = pc.tile([1, 2, qcw], f32, name="r2", tag="r2")
                    nc.vector.reciprocal_approx_fast(r2, s2_)
                    c2 = pc.tile([1, 2, qcw], bf16, name="c2", tag="c2")
                    nc.vector.tensor_mul(c2, r2, gtt)
                    # single 1-input partition broadcast: each partition gets
                    # both coefficient rows (the only Pool compute; 1-input Q7
                    # ops are line-rate, 2-input ones are not)
                    bc2 = pt.tile([64, 2, qcw], bf16, name="bc2", tag="bc2")
                    nc.gpsimd.partition_broadcast(bc2, c2)
                    return bc2
                return run

            def _chain_b(h, qs, cts2, bc):
                def run():
                    bc2 = bc
                    t1 = pt.tile([64, qcw], bf16, name="t1", tag="t1")
                    t2 = pt.tile([64, qcw], bf16, name="t2", tag="t2")
                    nc.vector.tensor_mul(t1, cts2[0:64, 0, :], bc2[:, 0, :])
                    nc.vector.tensor_mul(t2, cts2[0:64, 1, :], bc2[:, 1, :])
                    o = po.tile([64, qcw], bf16, name="o", tag="o")
                    nc.vector.tensor_add(o, t1, t2)
                    nc.sync.dma_start(out_d[h * 64:(h + 1) * 64, qs], o)
                return run

            # delayed-ctx software pipeline: block bi runs chunk bi's
            # scores+exp (filling the e-ring) interleaved per key tile with
            # chunk bi-1's mask+ctx matmuls, which read fully materialized
            # e tiles -> no exp->mask->ctx chain latency on the PE, no
            # per-chunk fill/drain
            chunks = [(h, qc) for h in range(HPC) for qc in range(S // qcw)]
            e_store = {}
            ct_store = {}
            for bi in range(len(chunks) + 1):
                cur = chunks[bi] if bi < len(chunks) else None
                prev = chunks[bi - 1] if bi > 0 else None
                if cur is not None:
                    e_store[cur] = [None] * KT
                    ch, cqc = cur
                    cm, cpar = ch // 2, ch % 2
                    cksl = slice(64 * cpar, 64 * cpar + 64)
                if prev is not None:
                    ph, pqc = prev
                    pqs = slice(pqc * qcw, (pqc + 1) * qcw)
                    ctl = pctx.tile([65, qcw], f32, name="ctl", tag="ctxl")
                    ctg = pctx.tile([65, qcw], f32, name="ctg", tag="ctxg")
                    ct_store[prev] = (ctl, ctg)
                for t in range(KT):
                    if cur is not None:
                        ps = psc.tile([128, qcw], f32, name="ps", tag="sc")
                        for hlf in range(qcw // 512):
                            nc.tensor.matmul(
                                ps[:, hlf * 512:(hlf + 1) * 512],
                                lhsT=kt_sb[cksl, cm, t * 128:(t + 1) * 128],
                                rhs=qt_sb[cksl, cm, cqc * qcw + hlf * 512:
                                          cqc * qcw + (hlf + 1) * 512],
                                start=True, stop=True,
                            )
                        e = pe.tile([128, qcw], bf16, name="e", tag="e")
                        nc.scalar.activation(e, ps, AF.Exp)
                        e_store[cur][t] = e
                    if prev is not None:
                        st0, st1 = (t == 0), (t == KT - 1)
                        epv = e_store[prev][t]
                        el = pel.tile([128, qcw], bf16, name="el", tag="el")
                        nc.vector.tensor_mul(el, epv, msk_ts[t][:, pqs])
                        if use_em:
                            eg = pel.tile([128, qcw], bf16, name="eg", tag="el")
                            nc.vector.tensor_scalar_mul(eg, epv, em_sb[:, t:t + 1])
                        else:
                            eg = epv
                        ctl, ctg = ct_store[prev]
                        for hlf in range(qcw // 512):
                            h5 = slice(hlf * 512, (hlf + 1) * 512)
                            nc.tensor.matmul(ctl[:, h5], lhsT=v_ts[t][:, ph, :],
                                             rhs=el[:, h5], start=st0, stop=st1)
                            nc.tensor.matmul(ctg[:, h5], lhsT=v_ts[t][:, ph, :],
                                             rhs=eg[:, h5], start=st0, stop=st1)
                    # two-blocks-ago chunk's normalize/combine chain; in the
                    # final drain block (no scores stream) flush later so the
                    # chain's DMA-waiting recip doesn't block the mask muls
                    # that pace the drain's ctx matmuls
                    ta, tb = (2, 6) if cur is not None else (8, 12)
                    if t == ta and pending[0] is not None:
                        pending[1] = pending[1](pending[0]())
                        pending[0] = None
                    elif t == tb and pending[1] is not None and pending[0] is None:
                        pending[1]()
                        pending[1] = None
                if prev is not None:
                    # chain head: ACT copies release ctx PSUM; tiny DMAs move
                    # the denominator row 64 to partition 0 + fetch gate rows
                    ctl, ctg = ct_store.pop(prev)
                    cts2 = pt.tile([65, 2, qcw], f32, name="cts2", tag="cts")
                    nc.scalar.activation(cts2[:, 0, :], ctl, AF.Copy)
                    nc.scalar.activation(cts2[:, 1, :], ctg, AF.Copy)
                    gtt = pc.tile([1, 2, qcw], bf16, name="gtt", tag="gtt")
                    nc.sync.dma_start(gtt, gt_d[ph, :, pqs])
                    s2_ = pc.tile([1, 2, qcw], f32, name="s2_", tag="s2")
                    nc.sync.dma_start(s2_, cts2[64:65, :, :])
                    pending[0] = _chain_a(cts2, s2_, gtt)
                    pending[1] = lambda bc, h=ph, qs=pqs, cts2=cts2: _chain_b(h, qs, cts2, bc)
                    del e_store[prev]
            if pending[0] is not None:
                pending[1] = pending[1](pending[0]())
                pending[0] = None
            if pending[1] is not None:
                pending[1]()
                pending[1] = None
            att_ctx.close()

    nc.compile()
    return nc


def _get(use_em: bool, has_b: bool):
    key = (use_em, has_b)
    if key not in _BUILT:
        _BUILT[key] = _build(use_em, has_b=has_b)
    return _BUILT[key]


def _prep_core(c, hs, am, lm, go, Wq, bq, Wk, bk, Wv, bv, use_em):
    bf = ml_dtypes.bfloat16
    b, hg = c // 4, c % 4
    h0 = hg * HPC
    sl = slice(h0 * DH, (h0 + HPC) * DH)
    g = go[b, h0:h0 + HPC, :, 0]                     # [HPC, S]
    m = {
        "xt": np.ascontiguousarray(hs[b].T).astype(bf),
        "wq": np.ascontiguousarray((Wq[sl, :] / 8.0).T).astype(bf),
        "wk": np.ascontiguousarray(Wk[sl, :].T).astype(bf),
        "wv": np.ascontiguousarray(Wv[sl, :].T).astype(bf),
        "bqk": np.stack([bq[sl] / 8.0, bk[sl]]).astype(np.float32),
        "bv": bv[sl].reshape(1, 256).astype(bf),
        "msk": np.ascontiguousarray(
            lm[b, 0].astype(np.float32).T).reshape(KT, 128, S).astype(bf),
        "gt": np.stack([g, 1.0 - g], axis=1).astype(bf),
    }
    if use_em:
        m["em"] = np.exp(am[b, 0, 0]).astype(np.float32).reshape(KT, 128)
    return m


def make_in_maps(inputs):
    hs = np.asarray(inputs["hidden_states"], np.float32)
    am = np.asarray(inputs["attention_mask"], np.float32)
    lm = np.asarray(inputs["local_attention_mask"])
    go = np.asarray(inputs["gate_outputs"], np.float32)
    Wq = np.asarray(inputs["Wq"], np.float32)
    bq = np.asarray(inputs["bq"], np.float32)
    Wk = np.asarray(inputs["Wk"], np.float32)
    bk = np.asarray(inputs["bk"], np.float32)
    Wv = np.asarray(inputs["Wv"], np.float32)
    bv = np.asarray(inputs["bv"], np.float32)
    use_em = bool(np.any(am != 0.0))
    has_b = bool(np.any(bq != 0.0) or np.any(bk != 0.0) or np.any(bv != 0.0))
    maps = [
        _prep_core(c, hs, am, lm, go, Wq, bq, Wk, bk, Wv, bv, use_em)
        for c in range(NCORES)
    ]
    return maps, (use_em, has_b)


def assemble(results):
    out = np.empty((B, S, H), np.float32)
    for c in range(NCORES):
        b, hg = c // 4, c % 4
        sl = slice(hg * HPC * DH, (hg + 1) * HPC * DH)
        out[b, :, sl] = np.asarray(results[c]["out"]).astype(np.float32).T
    return out


def kernel(**inputs):
    from concourse import bass_utils

    maps, (use_em, has_b) = make_in_maps(inputs)
    nc = _get(use_em, has_b)
    res = bass_utils.run_bass_kernel_spmd(nc, maps, core_ids=list(range(NCORES)))
    return assemble(res.results)
